# revision 1
# baseline (speedup 1.0000x reference)
"""Trainium2 Bass kernel for nn_CrossAttention_7129645711362.

Sharding: data-parallel over batch b (2) x tensor-parallel over heads (16):
8 cores, each owns (1 batch, 4 heads). Per core:
  qT   = (Wq_cols * d^-0.5).T @ x.T          (256 x 1024)
  kT   = Wk_cols.T @ ctx.T                   (256 x 4096)
  v'   = ctx @ [Wv_cols | interleaved ones]  (4096 x 260), ones col per head
  S^T  = kT_h^T-slice @ qT_h  per kv-chunk   (128 x 1024)  == pre_doc (transposed)
  att  = exp(S^T + ds_bias)                  (ACT, per-partition bias = doc-sim)
  [o^T; Z] += v'_h^T @ att                   accumulated in PSUM over 32 chunks
  o^T /= Z  (reciprocal + K=1 broadcast matmul + DVE mul)
  partial_out = o^T.T @ Wout_rows            (1024 x 1024)
Host gathers: out = sum-over-head-groups(partial_out) + bout;
pre_doc = per-core (4,4096,1024) transposed back on host.
"""

import sys

sys.path.insert(0, "/opt/trn_rl_repo")

import numpy as np

# problem constants (hardcoded per contract)
B, N, D = 2, 1024, 1024
M, L = 4, 1024
KV = M * L  # 4096
H = 16
HD = D // H  # 64
HPC = 4  # heads per core
FPC = HPC * HD  # 256 features per core
VW = HPC * (HD + 1)  # 260: v tile width with ones column per head
NCORES = 8
KCH = KV // 128  # 32 kv chunks
F32 = None  # set after import

_CACHE = {}


def _build_program():
    import concourse.mybir as mybir
    import concourse.tile as tile
    from concourse import bacc

    f32 = mybir.dt.float32
    EXP = mybir.ActivationFunctionType.Exp

    nc = bacc.Bacc("TRN2", target_bir_lowering=False, debug=False)

    xT_d = nc.dram_tensor("xT", (D, N), f32, kind="ExternalInput").ap()
    ctxT_d = nc.dram_tensor("ctxT", (D, KV), f32, kind="ExternalInput").ap()
    wq_d = nc.dram_tensor("wq", (D, FPC), f32, kind="ExternalInput").ap()
    wk_d = nc.dram_tensor("wk", (D, FPC), f32, kind="ExternalInput").ap()
    wv_d = nc.dram_tensor("wv", (D, VW), f32, kind="ExternalInput").ap()
    wout_d = nc.dram_tensor("wout", (HPC, HD, D), f32, kind="ExternalInput").ap()
    dsT_d = nc.dram_tensor("dsT", (128, KCH), f32, kind="ExternalInput").ap()

    preT_d = nc.dram_tensor("preT", (HPC, KV, N), f32, kind="ExternalOutput").ap()
    pout_d = nc.dram_tensor("pout", (N, D), f32, kind="ExternalOutput").ap()

    with tile.TileContext(nc) as tc:
        with tc.tile_pool(name="persist", bufs=1) as persist:
            # persistent tiles
            qT_t = [persist.tile([128, N], f32, tag=f"qT{i}", name=f"qT{i}") for i in range(2)]
            kT_t = [persist.tile([128, KV], f32, tag=f"kT{i}", name=f"kT{i}") for i in range(2)]
            v_t = [persist.tile([128, VW], f32, tag=f"v{i}", name=f"v{i}") for i in range(KCH)]
            oT_t = [persist.tile([HD, N], f32, tag=f"oT{i}", name=f"oT{i}") for i in range(HPC)]
            ds_t = persist.tile([128, KCH], f32, tag="ds", name="ds")
            ones_t = persist.tile([128, HD], f32, tag="ones", name="ones")
            nc.sync.dma_start(ds_t[:], dsT_d[:])
            nc.any.memset(ones_t[:], 1.0)

            # ---- Phase 1: qT = wq.T @ xT ----
            with (
                tc.tile_pool(name="wq_pool", bufs=1) as wqp,
                tc.tile_pool(name="xt_pool", bufs=3) as xtp,
                tc.tile_pool(name="q_ps", bufs=1, space="PSUM") as qpsp,
            ):
                wq_t = [wqp.tile([128, FPC], f32, tag=f"wq{k}", name=f"wq{k}") for k in range(8)]
                for k in range(8):
                    nc.sync.dma_start(wq_t[k][:], wq_d[k * 128 : (k + 1) * 128, :])
                qps = [
                    [qpsp.tile([128, 512], f32, tag=f"qps{mi}{nh}", name=f"qps{mi}{nh}") for nh in range(2)]
                    for mi in range(2)
                ]
                for k in range(8):
                    xt = xtp.tile([128, N], f32, tag="xt", name=f"xt{k}")
                    nc.sync.dma_start(xt[:], xT_d[k * 128 : (k + 1) * 128, :])
                    for mi in range(2):
                        for nh in range(2):
                            nc.tensor.matmul(
                                qps[mi][nh][:],
                                wq_t[k][:, mi * 128 : (mi + 1) * 128],
                                xt[:, nh * 512 : (nh + 1) * 512],
                                start=(k == 0),
                                stop=(k == 7),
                            )
                for mi in range(2):
                    for nh in range(2):
                        nc.any.tensor_copy(qT_t[mi][:, nh * 512 : (nh + 1) * 512], qps[mi][nh][:])

            # ---- Phase 2: kT = wk.T @ ctxT ; v = ctxT.T @ wv ----
            with (
                tc.tile_pool(name="wkv_pool", bufs=1) as wkvp,
                tc.tile_pool(name="ct_pool", bufs=2) as ctp,
                tc.tile_pool(name="kv_ps", bufs=2, space="PSUM") as kvpsp,
            ):
                wk_t = [wkvp.tile([128, FPC], f32, tag=f"wk{k}", name=f"wk{k}") for k in range(8)]
                wv_t = [wkvp.tile([128, VW], f32, tag=f"wv{k}", name=f"wv{k}") for k in range(8)]
                for k in range(8):
                    nc.sync.dma_start(wk_t[k][:], wk_d[k * 128 : (k + 1) * 128, :])
                    nc.sync.dma_start(wv_t[k][:], wv_d[k * 128 : (k + 1) * 128, :])
                for g in range(8):
                    ct = []
                    for k in range(8):
                        c = ctp.tile([128, 512], f32, tag=f"ct{k}", name=f"ct{k}_{g}")
                        nc.sync.dma_start(c[:], ctxT_d[k * 128 : (k + 1) * 128, g * 512 : (g + 1) * 512])
                        ct.append(c)
                    for mi in range(2):
                        kps = kvpsp.tile([128, 512], f32, tag="kps", name=f"kps{g}{mi}")
                        for k in range(8):
                            nc.tensor.matmul(
                                kps[:],
                                wk_t[k][:, mi * 128 : (mi + 1) * 128],
                                ct[k][:],
                                start=(k == 0),
                                stop=(k == 7),
                            )
                        nc.any.tensor_copy(kT_t[mi][:, g * 512 : (g + 1) * 512], kps[:])
                    for sub in range(4):
                        ci = g * 4 + sub
                        vps = kvpsp.tile([128, VW], f32, tag="vps", name=f"vps{ci}")
                        for k in range(8):
                            nc.tensor.matmul(
                                vps[:],
                                ct[k][:, sub * 128 : (sub + 1) * 128],
                                wv_t[k][:],
                                start=(k == 0),
                                stop=(k == 7),
                            )
                        nc.any.tensor_copy(v_t[ci][:], vps[:])
                        ones_cols = v_t[ci].rearrange("p (g e) -> p g e", e=HD + 1)[:, :, HD : HD + 1]
                        nc.any.memset(ones_cols, 1.0)

            # ---- Phase 3: attention per head ----
            with (
                tc.tile_pool(name="att_pool", bufs=3) as attp,
                tc.tile_pool(name="pst_pool", bufs=3) as pstp,
                tc.tile_pool(name="z_pool", bufs=1) as zp,
                tc.tile_pool(name="p3_ps", bufs=2, space="PSUM") as p3ps,
            ):
                for h in range(HPC):
                    mi, hi = h // 2, h % 2
                    hs = slice(hi * 64, hi * 64 + 64)
                    ops = p3ps.tile([HD + 1, N], f32, tag="O", name=f"ops{h}")
                    for c in range(KCH):
                        sps = p3ps.tile([128, N], f32, tag="S", name=f"sps{h}_{c}")
                        for nh in range(2):
                            nc.tensor.matmul(
                                sps[:, nh * 512 : (nh + 1) * 512],
                                kT_t[mi][hs, c * 128 : (c + 1) * 128],
                                qT_t[mi][hs, nh * 512 : (nh + 1) * 512],
                                start=True,
                                stop=True,
                            )
                        pst = pstp.tile([128, N], f32, tag="pst", name=f"pst{h}_{c}")
                        nc.vector.tensor_copy(pst[:], sps[:])
                        nc.sync.dma_start(preT_d[h, c * 128 : (c + 1) * 128, :], pst[:])
                        att = attp.tile([128, N], f32, tag="att", name=f"att{h}_{c}")
                        nc.scalar.activation(att[:], sps[:], EXP, bias=ds_t[:, c : c + 1], scale=1.0)
                        for nh in range(2):
                            nc.tensor.matmul(
                                ops[:, nh * 512 : (nh + 1) * 512],
                                v_t[c][:, h * (HD + 1) : (h + 1) * (HD + 1)],
                                att[:, nh * 512 : (nh + 1) * 512],
                                start=(c == 0),
                                stop=(c == KCH - 1),
                            )
                    # normalize: oT_h = o / Z  (Z = row 64 of ops)
                    z = zp.tile([128, 2 * N], f32, tag="z", name=f"z{h}")
                    nc.vector.tensor_copy(z[64:65, 0:N], ops[64:65, :])
                    nc.vector.reciprocal(z[64:65, N : 2 * N], z[64:65, 0:N])
                    bps = p3ps.tile([64, N], f32, tag="O", name=f"bps{h}")
                    for nh in range(2):
                        nc.tensor.matmul(
                            bps[:, nh * 512 : (nh + 1) * 512],
                            ones_t[64:65, 0:64],
                            z[64:65, N + nh * 512 : N + (nh + 1) * 512],
                            start=True,
                            stop=True,
                        )
                    bsb = zp.tile([64, N], f32, tag="bsb", name=f"bsb{h}")
                    nc.any.tensor_copy(bsb[:], bps[:])
                    nc.vector.tensor_mul(oT_t[h][:], ops[0:64, :], bsb[:])

            # ---- Phase 4: partial_out = oT.T @ wout ----
            with (
                tc.tile_pool(name="wout_pool", bufs=1) as wop,
                tc.tile_pool(name="po_pool", bufs=3) as pop,
                tc.tile_pool(name="p4_ps", bufs=2, space="PSUM") as p4ps,
            ):
                wout_t = [wop.tile([HD, D], f32, tag=f"wo{h}", name=f"wo{h}") for h in range(HPC)]
                for h in range(HPC):
                    nc.sync.dma_start(wout_t[h][:], wout_d[h, :, :])
                for nm in range(8):
                    po = pop.tile([128, D], f32, tag="po", name=f"po{nm}")
                    for nd in range(2):
                        pps = p4ps.tile([128, 512], f32, tag="pps", name=f"pps{nm}{nd}")
                        for h in range(HPC):
                            nc.tensor.matmul(
                                pps[:],
                                oT_t[h][:, nm * 128 : (nm + 1) * 128],
                                wout_t[h][:, nd * 512 : (nd + 1) * 512],
                                start=(h == 0),
                                stop=(h == HPC - 1),
                            )
                        nc.any.tensor_copy(po[:, nd * 512 : (nd + 1) * 512], pps[:])
                    nc.sync.dma_start(pout_d[nm * 128 : (nm + 1) * 128, :], po[:])

    nc.compile()
    return nc


def _get_program():
    if "nc" not in _CACHE:
        _CACHE["nc"] = _build_program()
    return _CACHE["nc"]


def _prep_core_inputs(core, x, context, doc_similarities, Wq, Wkv, Wout, beta):
    bb = core // HPC
    hg = (core % HPC) * HPC  # first head of this core's group
    cols = slice(hg * HD, (hg + HPC) * HD)
    scale = np.float32(D**-0.5)  # exactly 2**-5

    Wk_full = Wkv[:, :D]
    Wv_full = Wkv[:, D:]

    wq_c = np.ascontiguousarray(Wq[:, cols].astype(np.float32) * scale)
    wk_c = np.ascontiguousarray(Wk_full[:, cols].astype(np.float32))
    wv_c = np.zeros((D, VW), np.float32)
    for j in range(HPC):
        wv_c[:, j * (HD + 1) : j * (HD + 1) + HD] = Wv_full[:, (hg + j) * HD : (hg + j + 1) * HD]
    wout_c = np.ascontiguousarray(
        np.stack([Wout[(hg + j) * HD : (hg + j + 1) * HD, :] for j in range(HPC)])
    ).astype(np.float32)

    xT_c = np.ascontiguousarray(x[bb].T.astype(np.float32))
    ctxT_c = np.ascontiguousarray(context[bb].reshape(KV, D).T.astype(np.float32))
    ds_c = (np.repeat(doc_similarities[bb].astype(np.float32), L) * np.float32(beta)).astype(np.float32)
    dsT_c = np.ascontiguousarray(ds_c.reshape(KCH, 128).T)

    return {
        "xT": xT_c,
        "ctxT": ctxT_c,
        "wq": wq_c,
        "wk": wk_c,
        "wv": wv_c,
        "wout": wout_c,
        "dsT": dsT_c,
    }


def _reference_numpy(x, context, doc_similarities, mask, context_mask, Wq, Wkv, Wout, bout, beta):
    """Exact numpy fallback (used only if masks are not all ones)."""
    b, n, d = x.shape
    h = H
    hd = d // h
    _, m, L_, _ = context.shape
    kv_len = m * L_
    q = (x @ Wq).reshape(b, n, h, hd).transpose(0, 2, 1, 3)
    ctx = context.reshape(b, kv_len, d)
    cmask = context_mask.reshape(b, kv_len)
    ds = np.repeat(doc_similarities[:, :, None], L_, axis=-1).reshape(b, kv_len)
    ds = ds[:, None, None, :] * beta
    kvp = ctx @ Wkv
    k, v = np.split(kvp, 2, axis=-1)
    k = k.reshape(b, kv_len, h, hd).transpose(0, 2, 3, 1)
    v = v.reshape(b, kv_len, h, hd).transpose(0, 2, 1, 3)
    pre = np.einsum("bhnd,bhdk->bhnk", q, k) * (d**-0.5)
    att = pre + ds
    cross = (mask[:, None, :, None] * cmask[:, None, None, :]).astype(np.float32)
    att = att * cross
    att = np.where(att == 0.0, -np.inf, att)
    att = att - att.max(axis=-1, keepdims=True)
    w = np.exp(att)
    w = w / w.sum(axis=-1, keepdims=True)
    out = np.einsum("bhnk,bhkd->bhnd", w, v).transpose(0, 2, 1, 3).reshape(b, n, d)
    out = out @ Wout + bout
    return out.astype(np.float32), pre.astype(np.float32)


def kernel(x, context, doc_similarities, mask, context_mask, Wq, Wkv, Wout, bout, beta, **_unused):
    x = np.asarray(x)
    context = np.asarray(context)
    doc_similarities = np.asarray(doc_similarities)
    mask = np.asarray(mask)
    context_mask = np.asarray(context_mask)
    Wq = np.asarray(Wq, dtype=np.float32)
    Wkv = np.asarray(Wkv, dtype=np.float32)
    Wout = np.asarray(Wout, dtype=np.float32)
    bout = np.asarray(bout, dtype=np.float32)
    beta = np.float32(np.asarray(beta))

    if not (mask.all() and context_mask.all()):
        return _reference_numpy(
            x.astype(np.float32), context.astype(np.float32), doc_similarities.astype(np.float32),
            mask, context_mask, Wq, Wkv, Wout, bout, beta,
        )

    from concourse.bass_utils import run_bass_kernel_spmd

    nc = _get_program()
    in_maps = [
        _prep_core_inputs(c, x, context, doc_similarities, Wq, Wkv, Wout, beta)
        for c in range(NCORES)
    ]
    res = run_bass_kernel_spmd(nc, in_maps, core_ids=list(range(NCORES)))

    out = np.zeros((B, N, D), np.float32)
    pre = np.empty((B, H, N, KV), np.float32)
    for c in range(NCORES):
        bb = c // HPC
        hg = (c % HPC) * HPC
        r = res.results[c]
        out[bb] += r["pout"]
        pre[bb, hg : hg + HPC] = r["preT"].transpose(0, 2, 1)
    out += bout
    return out, pre
